# revision 1
# baseline (speedup 1.0000x reference)
"""Trainium2 Bass kernel for 5x5x5 all-ones Conv3d (box filter), stride 1, pad 2.

Input x: (4, 1, 128, 256, 256) fp32, W: (1,1,5,5,5) all-ones.
Output:  (4, 1, 128, 256, 256) fp32.

Strategy (8 NeuronCores): shard batch(4) x H-halves(2) -> 8 shards. The
all-ones conv is separable into three 5-tap box sums (W, H, D).
Per core:
  - input shard [D=128, H=132 (128 + 2 halo each side), W=260 (256 + 2 zero-pad)]
  - D lives on SBUF partitions (full 128) -> 100% lane utilization.
  - W-axis 5-tap box sum on VectorE: prefix scan along the flattened free
    dim (tensor_tensor_scan) + one windowed-difference subtract.
  - D-axis sum via a banded all-ones matrix matmul (the clipped band
    encodes the 'same' zero-padding in D); the H-axis sum is folded into
    the SAME matmuls as 5 PSUM-accumulating matmuls whose rhs access
    patterns are shifted by one H row each -> TensorE does both D and H
    reductions in float32r mode (1 cycle/row; quantizes the moving operand
    to ~12 mantissa bits -> ~1.0e-4 norm relative error).
  - ScalarE evicts PSUM -> SBUF and copies the 4-row `a` halo between
    chunks (each input row is DMA'd and W-summed exactly once).
  - HWDGE DMA: input tiles on the SP ring, output tiles on the ACT ring.
  - H is processed in tapered chunks [2,6,8,16x6,12,4] to shrink pipeline
    fill/drain.

Measured (8 concurrent cores, TRN2): ~93-100 us/core steady state via
REPEAT-differencing wall-clock; cost-model TimelineSim predicts 101.6 us.
Per-core DMA floor (17.9 MB in + 16.8 MB out at ~360 GB/s) is ~95 us, so
the kernel runs at the memory roofline. Relative error 1.04e-4 (fp32r).
"""

import numpy as np

import concourse.mybir as mybir
import concourse.tile as tile
from concourse import bacc
from concourse.bass_utils import run_bass_kernel_spmd

# Problem geometry (hardcoded; kernel.py must be self-contained).
B = 4
DEP = 128                  # depth (on partitions)
HGT = 256                  # height
WID = 256                  # width
KS = 5
R = 2                      # conv radius

N_CORES = 8
H_HALF = HGT // 2          # 128 output rows per core
H_IN = H_HALF + 2 * R      # 132 input rows per core
W_PAD = WID + 2 * R        # 260

HC = 16                    # main chunk output rows
# tapered chunk sizes (sum = H_HALF) to shrink pipeline fill/drain
CHUNKS = [2, 6, 8] + [16] * 6 + [12, 4]
ROWS_IN = HC + 2 * R       # max a-tile rows per chunk
XT_ROWS = HC               # max newly-loaded rows per chunk
ROWS_PER_SET = 512 // WID  # 2 output rows per PSUM bank (N = 512 fp32)

# Tunables
W_SUM_MODE = "scan"        # "s2" (3 vector ops) | "scan" (scan + subtract)
MM_DTYPE = "f32r"          # "f32r" | "f32" | "bf16"
REPEAT = 1                 # run the whole body N times (benchmarking only)
TRACE = False              # set True (from test.py) to profile
LAST_RESULT = None         # BassKernelResults of the last run (for test.py)

_NC_CACHE = {}


def _nonce_cols():
    key = (REPEAT, W_SUM_MODE, MM_DTYPE, tuple(CHUNKS), 3)
    return 8 + hash(key) % 4093


def _build_nc():
    """Build the per-core Bass program (identical on all 8 cores)."""
    nc = bacc.Bacc("TRN2", target_bir_lowering=False, debug=False)

    mm_store_dt = {
        "f32r": mybir.dt.float32r,
        "f32": mybir.dt.float32,
        "bf16": mybir.dt.bfloat16,
    }[MM_DTYPE]

    x_d = nc.dram_tensor("x", [DEP, H_IN, W_PAD], mybir.dt.float32,
                         kind="ExternalInput")
    band_d = nc.dram_tensor("band", [DEP, DEP], mm_store_dt,
                            kind="ExternalInput")
    # unused input whose shape encodes the config -> distinct HLO fingerprint
    # per kernel variant (defeats any shape-keyed executable caching)
    nc.dram_tensor("nonce", [1, _nonce_cols()], mybir.dt.float32,
                   kind="ExternalInput")
    y_d = nc.dram_tensor("y", [DEP, H_HALF, WID], mybir.dt.float32,
                         kind="ExternalOutput")

    with tile.TileContext(nc) as tc:
        with (
            tc.tile_pool(name="const", bufs=1) as cpool,
            tc.tile_pool(name="xin", bufs=3) as xin_pool,
            tc.tile_pool(name="tmp", bufs=1) as tmp_pool,
            tc.tile_pool(name="apool", bufs=3) as a_pool,
            tc.tile_pool(name="opool", bufs=3) as out_pool,
            tc.tile_pool(name="psum", bufs=8, space="PSUM") as ps_pool,
        ):
            band = cpool.tile([DEP, DEP], mm_store_dt, name="band")
            nc.sync.dma_start(out=band[:], in_=band_d[:])

            if W_SUM_MODE == "scan":
                # persistent prefix-sum buffer; col 0 stays 0 forever
                p = cpool.tile([DEP, XT_ROWS * W_PAD + 5], mybir.dt.float32,
                               name="p")
                nc.vector.memset(p[:, 0:1], 0.0)

            a_prev = None
            prev_oc = 0
            h0 = 0
            for idx, oc in enumerate(CHUNKS * REPEAT):
                c = idx % len(CHUNKS)
                if c == 0:
                    a_prev = None
                    prev_oc = 0
                    h0 = 0
                first = c == 0
                a_rows = oc + 2 * R     # rows of `a` this chunk consumes
                # chunk 0 loads its leading halo too; later chunks only load
                # their `oc` new rows (prior halo rows are reused via a_prev)
                n_new = a_rows if first else oc
                src0 = 0 if first else h0 + 2 * R
                xt = xin_pool.tile([DEP, XT_ROWS, W_PAD], mybir.dt.float32,
                                   name="xt", tag="xt")
                nc.sync.dma_start(out=xt[:, 0:n_new, :],
                                  in_=x_d[:, src0:src0 + n_new, :])

                # ---- W-axis 5-tap box sum -> a [DEP, a_rows, WID] ----
                # a rows correspond to input rows [h0, h0 + a_rows)
                a = a_pool.tile([DEP, ROWS_IN, WID], mm_store_dt,
                                name="a", tag="a")
                if not first:
                    # leading 4 halo rows = trailing 4 rows of previous chunk
                    nc.scalar.copy(out=a[:, 0:2 * R, :],
                                   in_=a_prev[:, prev_oc:prev_oc + 2 * R, :])
                a_dst = a[:, 0:a_rows, :] if first \
                    else a[:, 2 * R:a_rows, :]

                if W_SUM_MODE == "s2":
                    s2 = tmp_pool.tile([DEP, ROWS_IN, W_PAD - 1],
                                       mybir.dt.float32, name="s2", tag="s2")
                    nc.vector.tensor_add(
                        out=s2[:, 0:n_new, :],
                        in0=xt[:, 0:n_new, 0:W_PAD - 1],
                        in1=xt[:, 0:n_new, 1:W_PAD])
                    s4 = tmp_pool.tile([DEP, ROWS_IN, W_PAD - 3],
                                       mybir.dt.float32, name="s4", tag="s4")
                    nc.vector.tensor_add(
                        out=s4[:, 0:n_new, :],
                        in0=s2[:, 0:n_new, 0:W_PAD - 3],
                        in1=s2[:, 0:n_new, 2:W_PAD - 1])
                    nc.vector.tensor_add(
                        out=a_dst, in0=s4[:, 0:n_new, 0:WID],
                        in1=xt[:, 0:n_new, 4:W_PAD])
                else:  # "scan": prefix sum along flattened free dim + diff
                    # p[1+k] = sum of first (k+1) new elements;
                    # box(r, w) = p[r*W_PAD + w + 5] - p[r*W_PAD + w]
                    nflat = n_new * W_PAD
                    xt_flat = xt[:, 0:n_new, :].rearrange("q a b -> q (a b)")
                    nc.vector.tensor_tensor_scan(
                        out=p[:, 1:nflat + 1],
                        data0=xt_flat,
                        data1=xt_flat,
                        initial=0.0,
                        op0=mybir.AluOpType.add,
                        op1=mybir.AluOpType.bypass,
                    )
                    p_hi = p[:, 5:5 + nflat].rearrange(
                        "q (r w) -> q r w", r=n_new, w=W_PAD)[:, :, 0:WID]
                    p_lo = p[:, 0:nflat].rearrange(
                        "q (r w) -> q r w", r=n_new, w=W_PAD)[:, :, 0:WID]
                    nc.vector.tensor_sub(out=a_dst, in0=p_hi, in1=p_lo)

                # ---- D-sum + H-sum via 5 accumulating matmuls ----
                out_t = out_pool.tile([DEP, HC, WID], mybir.dt.float32,
                                      name="out_t", tag="out_t")
                for s in range(oc // ROWS_PER_SET):
                    r0 = s * ROWS_PER_SET
                    ps = ps_pool.tile([DEP, ROWS_PER_SET, WID],
                                      mybir.dt.float32, name="ps", tag="ps")
                    for j in range(KS):
                        rhs = a[:, r0 + j:r0 + j + ROWS_PER_SET, :]
                        nc.tensor.matmul(
                            ps[:], band[:], rhs,
                            start=(j == 0), stop=(j == KS - 1))
                    nc.scalar.copy(out=out_t[:, r0:r0 + ROWS_PER_SET, :],
                                   in_=ps[:])
                # out-DMA on the ACT HWDGE ring (separate FIFO from in-DMAs)
                nc.scalar.dma_start(out=y_d[:, h0:h0 + oc, :],
                                    in_=out_t[:, 0:oc, :])
                a_prev = a
                prev_oc = oc
                h0 += oc

    return nc


def _get_nc():
    key = (W_SUM_MODE, MM_DTYPE, REPEAT, tuple(CHUNKS))
    if key not in _NC_CACHE:
        nc = _build_nc()
        nc.compile()
        _NC_CACHE[key] = nc
    return _NC_CACHE[key]


def _make_band(scale=1.0):
    i = np.arange(DEP)
    band = (np.abs(i[:, None] - i[None, :]) <= R).astype(np.float32) * scale
    if MM_DTYPE == "bf16":
        import ml_dtypes
        band = band.astype(ml_dtypes.bfloat16)
    return np.ascontiguousarray(band)


def kernel(x, W=None, **_unused):
    global LAST_RESULT
    x = np.asarray(x, dtype=np.float32).reshape(B, DEP, HGT, WID)

    scale = 1.0
    if W is not None:
        scale = float(np.asarray(W, dtype=np.float32).ravel()[0])

    band = _make_band(scale)

    # Host-side shard: pad H and W by R with zeros, slice H halves with halo.
    nonce = np.zeros((1, _nonce_cols()), dtype=np.float32)
    in_maps = []
    for c in range(N_CORES):
        b, half = divmod(c, 2)
        xp = np.pad(x[b], ((0, 0), (R, R), (R, R)))  # (128, 260, 260)
        h_start = half * H_HALF
        shard = np.ascontiguousarray(xp[:, h_start:h_start + H_IN, :])
        in_maps.append({"x": shard, "band": band, "nonce": nonce})

    nc = _get_nc()
    res = run_bass_kernel_spmd(
        nc, in_maps, core_ids=list(range(N_CORES)), trace=TRACE)
    LAST_RESULT = res

    out = np.empty((B, 1, DEP, HGT, WID), dtype=np.float32)
    for c in range(N_CORES):
        b, half = divmod(c, 2)
        h_start = half * H_HALF
        out[b, 0, :, h_start:h_start + H_HALF, :] = res.results[c]["y"]
    return out

